# revision 15
# baseline (speedup 1.0000x reference)
"""Trainium2 Bass kernel for L2P top-k prompt selection (topk_masking).

Reference computation:
    nk  = l2_normalize(K, axis=1)                 # [30, 768]
    sim = l2_normalize(x_query) @ nk.T            # [8192, 30]
    idx = top_k(sim, 5)                           # [8192, 5]
    sel = p[idx]                                  # [8192, 5, 20, 768]
    Ek  = sel[:, :, :10, :].reshape(B, 50, 768)
    Ev  = sel[:, :, 10:, :].reshape(B, 50, 768)
    out = stack([Ek, Ev])                         # [2, 8192, 50, 768]

Strategy (8 cores, data-parallel over batch):
  - query normalization is skipped: top-k ranking is invariant to positive
    per-row scaling of the query.
  - scores = xq @ nk.T on TensorE (xq transposed on-chip via identity matmuls)
  - top-5 via DVE max8/max_index (ties resolved to lowest index, matching
    jax.lax.top_k)
  - gather p rows via fp16 one-hot matmuls. The one-hot selects a single
    fp16(p) value, so the only error is the fp16 quantization of p
    (~2.4e-4 L2 rel, far under the 2e-2 gate). p is cast f32->f16 during
    the input DMA (SWDGE cast), so there is no on-chip split work.
  - Ek/Ev halves are staged in separate [128, 7680] SBUF buffers (4-deep
    pool) so the Ek output DMA issues after 15 chunks and the output queue
    never starves. Output writes are the roofline: 315 MB/core at the
    ~358 GB/s HBM-per-NC limit.
"""

import sys
import types

import numpy as np

_B = 8192
_DK = 768
_D = 768
_POOL = 30
_PLEN = 20
_TOPK = 5
_NCORES = 8
_BSH = _B // _NCORES          # 1024 batch rows per core
_P = 128
_NTILES = _BSH // _P          # 8 tiles of 128 rows
_ROW = _PLEN * _D             # 15360 floats per selected prompt
_HALF = _ROW // 2             # 7680 (Ek / Ev halves)
_CHUNK = 512
_NCH = _ROW // _CHUNK         # 30 psum chunks per (tile, slot)
_NCHH = _NCH // 2             # 15 chunks per half


def _install_axon_hooks():
    """Make trace=True work under axon (profiling); harmless if absent."""
    if "antenv.axon_hooks" in sys.modules:
        return
    try:
        import trn_agent_boot.trn_boot as _tb

        hook = _tb._ntff_profile_via_ctypes("/opt/axon/libaxon_pjrt.so")
    except Exception:
        hook = None
    m = types.ModuleType("antenv.axon_hooks")
    m.get_axon_ntff_profile_hook = lambda: hook
    m.set_axon_ntff_profile_hook = lambda h: None
    sys.modules["antenv.axon_hooks"] = m


def build_bass():
    import concourse.bacc as bacc
    import concourse.mybir as mybir
    import concourse.tile as tile
    from concourse.masks import make_identity

    f32 = mybir.dt.float32
    f16 = mybir.dt.float16
    nc = bacc.Bacc(None, target_bir_lowering=False)

    xq_d = nc.dram_tensor("xq", [_BSH, _DK], f32, kind="ExternalInput")
    k_d = nc.dram_tensor("kk", [_POOL, _DK], f32, kind="ExternalInput")
    p_d = nc.dram_tensor("pp", [_POOL, _ROW], f32, kind="ExternalInput")
    out_d = nc.dram_tensor("out", [2, _BSH, _TOPK, _HALF], f32, kind="ExternalOutput")

    with tile.TileContext(nc) as tc:
        with (
            tc.tile_pool(name="const", bufs=1) as cpool,
            tc.tile_pool(name="xq", bufs=6) as xqpool,
            tc.tile_pool(name="xqt", bufs=2) as xqtpool,
            tc.tile_pool(name="topk", bufs=2) as tkpool,
            tc.tile_pool(name="oht", bufs=2) as ohtpool,
            tc.tile_pool(name="stage", bufs=4) as stpool,
            tc.tile_pool(name="ps_small", bufs=1, space="PSUM") as pss,
            tc.tile_pool(name="ps_trans", bufs=2, space="PSUM") as pst,
            tc.tile_pool(name="ps_gather", bufs=5, space="PSUM") as psg,
        ):
            # ---- constants and pool-side tensors ----
            ident = cpool.tile([_P, _P], f32)
            make_identity(nc, ident[:])

            iota_i = cpool.tile([_P, _POOL], mybir.dt.int32)
            nc.gpsimd.iota(iota_i[:], [[1, _POOL]], channel_multiplier=0)
            iota_f = cpool.tile([_P, _POOL], f32)
            nc.vector.tensor_copy(iota_f[:], iota_i[:])

            # SWDGE input queue order matters: it is FIFO, so the small k and
            # xq0 loads go ahead of the slow p16 cast-DMA (which would delay
            # tile 0's transposes by ~10us otherwise). p16 is split per
            # Ek/Ev column half so the first gather chunks wait only on the
            # first half.
            xq_tiles = []
            for i in range(_NTILES):
                xq_sb = xqpool.tile([_P, _DK], f32, tag="xq")
                xq_tiles.append(xq_sb)
            nc.gpsimd.dma_start(out=xq_tiles[0][:], in_=xq_d[:_P, :])

            k_sb = cpool.tile([_POOL, _DK], f32)
            nc.gpsimd.dma_start(out=k_sb[:], in_=k_d[:])

            # p cast f32->f16 during the load DMA (SWDGE). The cast path is
            # slow (~100 GB/s), so split into column quarters: the first
            # gather chunks only gate on the first quarter.
            p16 = cpool.tile([_POOL, _ROW], f16)
            _Q = _ROW // 4
            for q in range(4):
                nc.gpsimd.dma_start(
                    out=p16[:, q * _Q : (q + 1) * _Q],
                    in_=p_d[:, q * _Q : (q + 1) * _Q],
                )

            for i in range(1, _NTILES):
                nc.gpsimd.dma_start(
                    out=xq_tiles[i][:], in_=xq_d[i * _P : (i + 1) * _P, :]
                )

            def emit_xqt(xq_sb):
                # transpose xq tile -> xqT chunks [128f, 128b]; psum slots come
                # from the (deep) gather pool so the transposes pipeline
                xqt = xqtpool.tile([_P, _DK], f32, tag="xqt")
                for j in range(6):
                    ps_t = pst.tile([_P, _P], f32, space="PSUM", tag="ps_t")
                    nc.tensor.transpose(
                        ps_t[:], xq_sb[:, j * _P : (j + 1) * _P], ident[:]
                    )
                    if j % 2 == 0:
                        nc.vector.tensor_copy(xqt[:, j * _P : (j + 1) * _P], ps_t[:])
                    else:
                        nc.scalar.copy(xqt[:, j * _P : (j + 1) * _P], ps_t[:])
                return xqt

            # tile 0's transposes run on PE while ACT/DVE do the nk chain
            xqt0 = emit_xqt(xq_tiles[0])

            # ---- normalize K rows: nk = K / ||K|| ----
            nk = cpool.tile([_POOL, _DK], f32)
            ss = cpool.tile([_POOL, 1], f32)
            # nk used as scratch for K^2; ss accumulates the row sums
            nc.scalar.activation(
                nk[:], k_sb[:], mybir.ActivationFunctionType.Square, accum_out=ss[:]
            )
            nrm = cpool.tile([_POOL, 1], f32)
            nc.scalar.activation(nrm[:], ss[:], mybir.ActivationFunctionType.Sqrt)
            inv = cpool.tile([_POOL, 1], f32)
            nc.vector.reciprocal(inv[:], nrm[:])
            nc.vector.tensor_scalar_mul(nk[:], k_sb[:], inv[:])

            # ---- nkT [768, 30] as 6 chunks of [128, 30] ----
            nkt = cpool.tile([_P, 6 * _POOL], f32)
            for j in range(6):
                ps_t = pst.tile([_P, _POOL], f32, space="PSUM", tag="ps_t")
                nc.tensor.transpose(
                    ps_t[:], nk[:, j * _P : (j + 1) * _P], ident[:_POOL, :_POOL]
                )
                nc.vector.tensor_copy(nkt[:, j * _POOL : (j + 1) * _POOL], ps_t[:])

            # ---- per batch tile ----
            for i in range(_NTILES):
                xqt = xqt0 if i == 0 else emit_xqt(xq_tiles[i])

                # scores [128b, 30] = sum_j xqT_j.T @ nkT_j
                ps_sc = pss.tile([_P, _POOL], f32, space="PSUM")
                for j in range(6):
                    nc.tensor.matmul(
                        ps_sc[:],
                        lhsT=xqt[:, j * _P : (j + 1) * _P],
                        rhs=nkt[:, j * _POOL : (j + 1) * _POOL],
                        start=(j == 0),
                        stop=(j == 5),
                    )
                sc = tkpool.tile([_P, _POOL], f32)
                nc.vector.tensor_copy(sc[:], ps_sc[:])

                # top-5 indices (ties -> lowest index, like jax.lax.top_k)
                mx = tkpool.tile([_P, 8], f32)
                mi = tkpool.tile([_P, 8], mybir.dt.uint32)
                nc.vector.max(mx[:], sc[:])
                nc.vector.max_index(mi[:], mx[:], sc[:])
                mif = tkpool.tile([_P, 8], f32)
                nc.vector.tensor_copy(mif[:], mi[:])

                # one-hots [128, 30] -> transposed [30, 128] fp16 for matmul lhsT
                oht = ohtpool.tile([_POOL, _TOPK * _P], f16)
                for t in range(_TOPK):
                    oh = tkpool.tile([_P, _POOL], f32)
                    nc.vector.tensor_tensor(
                        out=oh[:],
                        in0=iota_f[:],
                        in1=mif[:, t : t + 1].to_broadcast([_P, _POOL]),
                        op=mybir.AluOpType.is_equal,
                    )
                    ps_o = pst.tile([_POOL, _P], f32, space="PSUM", tag="ps_t")
                    nc.tensor.transpose(ps_o[:], oh[:], ident[:])
                    nc.vector.tensor_copy(oht[:, t * _P : (t + 1) * _P], ps_o[:])

                # gather: sel[b] = p[idx[b,t]] via fp16 one-hot matmuls,
                # staged per Ek/Ev half so the Ek DMA issues at half time
                for t in range(_TOPK):
                    stk = stpool.tile([_P, _HALF], f32, tag="st")
                    stv = stpool.tile([_P, _HALF], f32, tag="st")
                    for c in range(_NCH):
                        ps_g = psg.tile([_P, _CHUNK], f32, space="PSUM")
                        nc.tensor.matmul(
                            ps_g[:],
                            lhsT=oht[:, t * _P : (t + 1) * _P],
                            rhs=p16[:, c * _CHUNK : (c + 1) * _CHUNK],
                            start=True,
                            stop=True,
                        )
                        half = stk if c < _NCHH else stv
                        cc = c if c < _NCHH else c - _NCHH
                        dst = half[:, cc * _CHUNK : (cc + 1) * _CHUNK]
                        if c % 2 == 0:
                            nc.scalar.copy(dst, ps_g[:])
                        else:
                            nc.vector.tensor_copy(dst, ps_g[:])
                        if i == 0 and t == 0:
                            # first slot: split the Ek write so the output
                            # stream starts ~4us earlier
                            if c == 7:
                                nc.sync.dma_start(
                                    out=out_d[0, :_P, 0, : 8 * _CHUNK],
                                    in_=stk[:, : 8 * _CHUNK],
                                )
                            elif c == _NCHH - 1:
                                nc.sync.dma_start(
                                    out=out_d[0, :_P, 0, 8 * _CHUNK :],
                                    in_=stk[:, 8 * _CHUNK :],
                                )
                        elif c == _NCHH - 1:
                            nc.sync.dma_start(
                                out=out_d[0, i * _P : (i + 1) * _P, t, :],
                                in_=stk[:],
                            )
                    nc.sync.dma_start(
                        out=out_d[1, i * _P : (i + 1) * _P, t, :], in_=stv[:]
                    )

    nc.compile()
    return nc


_NC_CACHE = None


def _get_nc():
    global _NC_CACHE
    if _NC_CACHE is None:
        _install_axon_hooks()
        _NC_CACHE = build_bass()
    return _NC_CACHE


def kernel(x_query, x, K, p, layer_id, trace=False, tmpdir=None):
    from concourse.bass_utils import run_bass_kernel_spmd

    nc = _get_nc()

    x_query = np.ascontiguousarray(np.asarray(x_query, dtype=np.float32))
    K = np.ascontiguousarray(np.asarray(K, dtype=np.float32))
    p2 = np.ascontiguousarray(np.asarray(p, dtype=np.float32)).reshape(_POOL, _ROW)

    in_maps = []
    for c in range(_NCORES):
        in_maps.append(
            {
                "xq": x_query[c * _BSH : (c + 1) * _BSH],
                "kk": K,
                "pp": p2,
            }
        )

    kw = {}
    if trace:
        import concourse.bass_utils as bass_utils

        bass_utils.upload_artifacts = lambda d: d
        kw = {"trace": True, "tmpdir": tmpdir}
    res = run_bass_kernel_spmd(nc, in_maps, core_ids=list(range(_NCORES)), **kw)

    shards = [
        res.results[c]["out"].reshape(2, _BSH, _TOPK * (_PLEN // 2), _D)
        for c in range(_NCORES)
    ]
    out = np.concatenate(shards, axis=1)
    if trace:
        return out, res
    return out


# revision 18
# speedup vs baseline: 1.0489x; 1.0489x over previous
"""Trainium2 Bass kernel for L2P top-k prompt selection (topk_masking).

Reference computation:
    nk  = l2_normalize(K, axis=1)                 # [30, 768]
    sim = l2_normalize(x_query) @ nk.T            # [8192, 30]
    idx = top_k(sim, 5)                           # [8192, 5]
    sel = p[idx]                                  # [8192, 5, 20, 768]
    Ek  = sel[:, :, :10, :].reshape(B, 50, 768)
    Ev  = sel[:, :, 10:, :].reshape(B, 50, 768)
    out = stack([Ek, Ev])                         # [2, 8192, 50, 768]

Strategy (8 cores, data-parallel over batch):
  - query normalization is skipped: top-k ranking is invariant to positive
    per-row scaling of the query.
  - scores = xq @ nk.T on TensorE (xq transposed on-chip via identity matmuls)
  - top-5 via DVE max8/max_index (ties resolved to lowest index, matching
    jax.lax.top_k)
  - gather p rows via fp16 one-hot matmuls. The one-hot selects a single
    fp16(p) value, so the only error is the fp16 quantization of p
    (~2.4e-4 L2 rel, far under the 2e-2 gate). p is cast f32->f16 during
    the input DMA (SWDGE cast), so there is no on-chip split work.
  - Ek/Ev halves are staged in separate [128, 7680] SBUF buffers (4-deep
    pool) so the Ek output DMA issues after 15 chunks and the output queue
    never starves. Output writes are the roofline: 315 MB/core at the
    ~358 GB/s HBM-per-NC limit.
"""

import sys
import types

import numpy as np

_B = 8192
_DK = 768
_D = 768
_POOL = 30
_PLEN = 20
_TOPK = 5
_NCORES = 8
_BSH = _B // _NCORES          # 1024 batch rows per core
_P = 128
_NTILES = _BSH // _P          # 8 tiles of 128 rows
_ROW = _PLEN * _D             # 15360 floats per selected prompt
_HALF = _ROW // 2             # 7680 (Ek / Ev halves)
_CHUNK = 512
_NCH = _ROW // _CHUNK         # 30 psum chunks per (tile, slot)
_NCHH = _NCH // 2             # 15 chunks per half


def _install_axon_hooks():
    """Make trace=True work under axon (profiling); harmless if absent."""
    if "antenv.axon_hooks" in sys.modules:
        return
    try:
        import trn_agent_boot.trn_boot as _tb

        hook = _tb._ntff_profile_via_ctypes("/opt/axon/libaxon_pjrt.so")
    except Exception:
        hook = None
    m = types.ModuleType("antenv.axon_hooks")
    m.get_axon_ntff_profile_hook = lambda: hook
    m.set_axon_ntff_profile_hook = lambda h: None
    sys.modules["antenv.axon_hooks"] = m


def build_bass():
    import concourse.bacc as bacc
    import concourse.mybir as mybir
    import concourse.tile as tile
    from concourse.masks import make_identity

    f32 = mybir.dt.float32
    f16 = mybir.dt.float16
    nc = bacc.Bacc(None, target_bir_lowering=False)

    xq_d = nc.dram_tensor("xq", [_BSH, _DK], f32, kind="ExternalInput")
    k_d = nc.dram_tensor("kk", [_POOL, _DK], f32, kind="ExternalInput")
    # p viewed as [120, 3840] so the raw f32 load spreads over 120 partitions
    p_d = nc.dram_tensor("pp", [4 * _POOL, _ROW // 4], f32, kind="ExternalInput")
    out_d = nc.dram_tensor("out", [2, _BSH, _TOPK, _HALF], f32, kind="ExternalOutput")

    with tile.TileContext(nc) as tc:
        with (
            tc.tile_pool(name="const", bufs=1) as cpool,
            tc.tile_pool(name="xq", bufs=6) as xqpool,
            tc.tile_pool(name="xqt", bufs=2) as xqtpool,
            tc.tile_pool(name="topk", bufs=2) as tkpool,
            tc.tile_pool(name="oht", bufs=2) as ohtpool,
            tc.tile_pool(name="stage", bufs=4) as stpool,
            tc.tile_pool(name="ps_small", bufs=1, space="PSUM") as pss,
            tc.tile_pool(name="ps_trans", bufs=2, space="PSUM") as pst,
            tc.tile_pool(name="ps_gather", bufs=5, space="PSUM") as psg,
        ):
            # ---- constants and pool-side tensors ----
            ident = cpool.tile([_P, _P], f32)
            make_identity(nc, ident[:])

            iota_i = cpool.tile([_P, _POOL], mybir.dt.int32)
            nc.gpsimd.iota(iota_i[:], [[1, _POOL]], channel_multiplier=0)
            iota_f = cpool.tile([_P, _POOL], f32)
            nc.vector.tensor_copy(iota_f[:], iota_i[:])

            # SWDGE input queue order matters: it is FIFO, so the small k and
            # xq0 loads go ahead of the slow p16 cast-DMA (which would delay
            # tile 0's transposes by ~10us otherwise). p16 is split per
            # Ek/Ev column half so the first gather chunks wait only on the
            # first half.
            xq_tiles = []
            for i in range(_NTILES):
                xq_sb = xqpool.tile([_P, _DK], f32, tag="xq")
                xq_tiles.append(xq_sb)
            nc.gpsimd.dma_start(out=xq_tiles[0][:], in_=xq_d[:_P, :])

            k_sb = cpool.tile([_POOL, _DK], f32)
            nc.gpsimd.dma_start(out=k_sb[:], in_=k_d[:])

            # preload the Sqrt ACT table early (dummy op) so the K-norm chain
            # doesn't serialize a ~1.3us table load mid-chain
            sq_dummy = cpool.tile([1, 1], f32)
            nc.scalar.activation(
                sq_dummy[:], ident[:1, :1], mybir.ActivationFunctionType.Sqrt
            )

            # p -> f16: the SWDGE cast-DMA has ~14us fixed latency, so instead
            # load raw f32 as [120, 3840] via fast HWDGE (borrowing two stage
            # slots), cast on DVE at full partition parallelism, then one
            # SBUF->SBUF DMA remaps into the [30, 15360] gather layout.
            p16 = cpool.tile([_POOL, _ROW], f16)
            p32v = stpool.tile([_P, _ROW // 4], f32, tag="st")
            nc.sync.dma_start(out=p32v[: 4 * _POOL, :], in_=p_d[:])
            p16v = stpool.tile([_P, _ROW // 4], f16, tag="st")
            nc.vector.tensor_copy(p16v[: 4 * _POOL, :], p32v[: 4 * _POOL, :])
            nc.sync.dma_start(out=p16[:], in_=p16v[: 4 * _POOL, :])

            for i in range(1, _NTILES):
                nc.gpsimd.dma_start(
                    out=xq_tiles[i][:], in_=xq_d[i * _P : (i + 1) * _P, :]
                )

            def emit_xqt(xq_sb):
                # transpose xq tile -> xqT chunks [128f, 128b]; psum slots come
                # from the (deep) gather pool so the transposes pipeline
                xqt = xqtpool.tile([_P, _DK], f32, tag="xqt")
                for j in range(6):
                    ps_t = pst.tile([_P, _P], f32, space="PSUM", tag="ps_t")
                    nc.tensor.transpose(
                        ps_t[:], xq_sb[:, j * _P : (j + 1) * _P], ident[:]
                    )
                    if j % 2 == 0:
                        nc.vector.tensor_copy(xqt[:, j * _P : (j + 1) * _P], ps_t[:])
                    else:
                        nc.scalar.copy(xqt[:, j * _P : (j + 1) * _P], ps_t[:])
                return xqt

            # tile 0's transposes run on PE while ACT/DVE do the nk chain
            xqt0 = emit_xqt(xq_tiles[0])

            # ---- normalize K rows: nk = K / ||K|| ----
            nk = cpool.tile([_POOL, _DK], f32)
            ss = cpool.tile([_POOL, 1], f32)
            # nk used as scratch for K^2; ss accumulates the row sums
            nc.scalar.activation(
                nk[:], k_sb[:], mybir.ActivationFunctionType.Square, accum_out=ss[:]
            )
            nrm = cpool.tile([_POOL, 1], f32)
            nc.scalar.activation(nrm[:], ss[:], mybir.ActivationFunctionType.Sqrt)
            inv = cpool.tile([_POOL, 1], f32)
            nc.vector.reciprocal(inv[:], nrm[:])
            nc.vector.tensor_scalar_mul(nk[:], k_sb[:], inv[:])

            # ---- nkT [768, 30] as 6 chunks of [128, 30] ----
            nkt = cpool.tile([_P, 6 * _POOL], f32)
            for j in range(6):
                ps_t = pst.tile([_P, _POOL], f32, space="PSUM", tag="ps_t")
                nc.tensor.transpose(
                    ps_t[:], nk[:, j * _P : (j + 1) * _P], ident[:_POOL, :_POOL]
                )
                nc.vector.tensor_copy(nkt[:, j * _POOL : (j + 1) * _POOL], ps_t[:])

            # ---- per batch tile ----
            for i in range(_NTILES):
                xqt = xqt0 if i == 0 else emit_xqt(xq_tiles[i])

                # scores [128b, 30] = sum_j xqT_j.T @ nkT_j
                ps_sc = pss.tile([_P, _POOL], f32, space="PSUM")
                for j in range(6):
                    nc.tensor.matmul(
                        ps_sc[:],
                        lhsT=xqt[:, j * _P : (j + 1) * _P],
                        rhs=nkt[:, j * _POOL : (j + 1) * _POOL],
                        start=(j == 0),
                        stop=(j == 5),
                    )
                sc = tkpool.tile([_P, _POOL], f32)
                nc.vector.tensor_copy(sc[:], ps_sc[:])

                # top-5 indices (ties -> lowest index, like jax.lax.top_k)
                mx = tkpool.tile([_P, 8], f32)
                mi = tkpool.tile([_P, 8], mybir.dt.uint32)
                nc.vector.max(mx[:], sc[:])
                nc.vector.max_index(mi[:], mx[:], sc[:])
                mif = tkpool.tile([_P, 8], f32)
                nc.vector.tensor_copy(mif[:], mi[:])

                # one-hots [128, 30] -> transposed [30, 128] fp16 for matmul lhsT
                oht = ohtpool.tile([_POOL, _TOPK * _P], f16)
                for t in range(_TOPK):
                    oh = tkpool.tile([_P, _POOL], f32)
                    nc.vector.tensor_tensor(
                        out=oh[:],
                        in0=iota_f[:],
                        in1=mif[:, t : t + 1].to_broadcast([_P, _POOL]),
                        op=mybir.AluOpType.is_equal,
                    )
                    ps_o = pst.tile([_POOL, _P], f32, space="PSUM", tag="ps_t")
                    nc.tensor.transpose(ps_o[:], oh[:], ident[:])
                    nc.vector.tensor_copy(oht[:, t * _P : (t + 1) * _P], ps_o[:])

                # gather: sel[b] = p[idx[b,t]] via fp16 one-hot matmuls,
                # staged per Ek/Ev half so the Ek DMA issues at half time
                for t in range(_TOPK):
                    stk = stpool.tile([_P, _HALF], f32, tag="st")
                    stv = stpool.tile([_P, _HALF], f32, tag="st")
                    for c in range(_NCH):
                        ps_g = psg.tile([_P, _CHUNK], f32, space="PSUM")
                        nc.tensor.matmul(
                            ps_g[:],
                            lhsT=oht[:, t * _P : (t + 1) * _P],
                            rhs=p16[:, c * _CHUNK : (c + 1) * _CHUNK],
                            start=True,
                            stop=True,
                        )
                        half = stk if c < _NCHH else stv
                        cc = c if c < _NCHH else c - _NCHH
                        dst = half[:, cc * _CHUNK : (cc + 1) * _CHUNK]
                        if c % 2 == 0:
                            nc.scalar.copy(dst, ps_g[:])
                        else:
                            nc.vector.tensor_copy(dst, ps_g[:])
                        if i == 0 and t == 0:
                            # first slot: split the Ek write so the output
                            # stream starts ~4us earlier
                            if c == 7:
                                nc.sync.dma_start(
                                    out=out_d[0, :_P, 0, : 8 * _CHUNK],
                                    in_=stk[:, : 8 * _CHUNK],
                                )
                            elif c == _NCHH - 1:
                                nc.sync.dma_start(
                                    out=out_d[0, :_P, 0, 8 * _CHUNK :],
                                    in_=stk[:, 8 * _CHUNK :],
                                )
                        elif c == _NCHH - 1:
                            nc.sync.dma_start(
                                out=out_d[0, i * _P : (i + 1) * _P, t, :],
                                in_=stk[:],
                            )
                    nc.sync.dma_start(
                        out=out_d[1, i * _P : (i + 1) * _P, t, :], in_=stv[:]
                    )

    nc.compile()
    return nc


_NC_CACHE = None


def _get_nc():
    global _NC_CACHE
    if _NC_CACHE is None:
        _install_axon_hooks()
        _NC_CACHE = build_bass()
    return _NC_CACHE


def kernel(x_query, x, K, p, layer_id, trace=False, tmpdir=None):
    from concourse.bass_utils import run_bass_kernel_spmd

    nc = _get_nc()

    x_query = np.ascontiguousarray(np.asarray(x_query, dtype=np.float32))
    K = np.ascontiguousarray(np.asarray(K, dtype=np.float32))
    p2 = np.ascontiguousarray(np.asarray(p, dtype=np.float32)).reshape(
        4 * _POOL, _ROW // 4
    )

    in_maps = []
    for c in range(_NCORES):
        in_maps.append(
            {
                "xq": x_query[c * _BSH : (c + 1) * _BSH],
                "kk": K,
                "pp": p2,
            }
        )

    kw = {}
    if trace:
        import concourse.bass_utils as bass_utils

        bass_utils.upload_artifacts = lambda d: d
        kw = {"trace": True, "tmpdir": tmpdir}
    res = run_bass_kernel_spmd(nc, in_maps, core_ids=list(range(_NCORES)), **kw)

    shards = [
        res.results[c]["out"].reshape(2, _BSH, _TOPK * (_PLEN // 2), _D)
        for c in range(_NCORES)
    ]
    out = np.concatenate(shards, axis=1)
    if trace:
        return out, res
    return out
